# revision 4
# baseline (speedup 1.0000x reference)
"""VQ codebook-quantizer kernel for Trainium2 (8 NeuronCores, SPMD data-parallel).

Problem: x [64,64,32,32] (NCHW, C=embed dim 64), codebook emb [1024,64].
Per token t (one (b,h,w) position): idx = argmin_k ||x_t - e_k||^2,
quantized = e_idx (straight-through => output is exactly the gathered row),
loss = 1.25 * mean((quant - x)^2).

Strategy (exact fp32 argmin, zero-flip vs a jax fp32 reference):
 - shard batch dim across 8 cores (8 batches/core = 8192 tokens/core)
 - NCHW means x[b] is already [D=64, HW=1024] = tokens-in-free-dim; use it
   directly as matmul lhsT with an appended ones-row so PE computes
   s[t,k] = x_t . e_k - ||e_k||^2/2   (argmax_k s = argmin_k dist)
 - DVE max (top-8) + max_index give exact fp32 argmax + index per token
 - indirect DMA gathers emb rows; PE transpose puts them back into NCHW
 - loss uses sum(max) : ||x_t - e_k*||^2 = ||x_t||^2 - 2*max_t
"""

import numpy as np

N_CORES = 8
B, D, H, W = 64, 64, 32, 32
HW = H * W
K = 1024
BPC = B // N_CORES   # batches per core
TPB = HW // 128      # 128-token tiles per batch
COMMITMENT_COST = 0.25

_CACHE = {}


def _build_program(repeat=1):
    """Build + compile the per-core Bass/Tile program (identical on all cores).

    repeat>1 re-runs the whole body (same outputs) for steady-state timing:
    device time per iteration = (wall(R) - wall(1)) / (R - 1).
    """
    import concourse.bass as bass
    import concourse.tile as tile
    from concourse import bacc, mybir
    from concourse.masks import make_identity

    f32 = mybir.dt.float32

    nc = bacc.Bacc("TRN2", target_bir_lowering=False, debug=False,
                   num_devices=N_CORES)

    x_d = nc.dram_tensor("x", [BPC, D, HW], f32, kind="ExternalInput").ap()
    emba_d = nc.dram_tensor("emba", [D + 1, K], f32, kind="ExternalInput").ap()
    embg_d = nc.dram_tensor("embg", [K, D], f32, kind="ExternalInput").ap()
    quant_d = nc.dram_tensor("quant", [BPC, D, HW], f32, kind="ExternalOutput").ap()
    msum_d = nc.dram_tensor("msum", [128, 1], f32, kind="ExternalOutput").ap()

    with tile.TileContext(nc) as tc:
        with (
            tc.tile_pool(name="const", bufs=1) as constp,
            tc.tile_pool(name="xb", bufs=2) as xbp,
            tc.tile_pool(name="ps", bufs=2, space="PSUM") as psp,
            tc.tile_pool(name="pst", bufs=2, space="PSUM") as pstp,
            tc.tile_pool(name="idx", bufs=4) as idxp,
            tc.tile_pool(name="g", bufs=4) as gp,
            tc.tile_pool(name="stage", bufs=2) as stagep,
        ):
            ident = constp.tile([128, 128], f32)
            make_identity(nc, ident[:])
            emba = constp.tile([D + 1, K], f32)
            nc.sync.dma_start(emba[:], emba_d[:])
            # top-8 per token tile, laid out side by side; column 8*i is the max
            tops = constp.tile([128, 8 * TPB * BPC], f32)

            for _rep in range(repeat):
                _kernel_body(nc, bass, mybir, tc, xbp, psp, pstp, idxp, gp,
                             stagep, ident, emba, tops, x_d, emba_d, embg_d,
                             quant_d)

            msum = constp.tile([128, 1], f32)
            tview = tops[:].rearrange("p (n e) -> p n e", e=8)[:, :, 0:1]
            nc.vector.tensor_reduce(out=msum[:], in_=tview,
                                    axis=mybir.AxisListType.XY,
                                    op=mybir.AluOpType.add)
            nc.sync.dma_start(msum_d[:], msum[:])

    nc.compile()
    return nc


def _kernel_body(nc, bass, mybir, tc, xbp, psp, pstp, idxp, gp, stagep,
                 ident, emba, tops, x_d, emba_d, embg_d, quant_d):
    f32 = mybir.dt.float32
    if True:
            for b in range(BPC):
                xb = xbp.tile([D + 1, HW], f32)
                nc.sync.dma_start(xb[0:D, :], x_d[b])
                nc.vector.memset(xb[D:D + 1, :], 1.0)
                stage = stagep.tile([D, HW], f32)
                for t in range(TPB):
                    i = b * TPB + t
                    sp = psp.tile([128, K], f32)
                    lhsT = xb[:, t * 128:(t + 1) * 128]
                    nc.tensor.matmul(sp[:, 0:512], lhsT=lhsT,
                                     rhs=emba[:, 0:512], start=True, stop=True)
                    nc.tensor.matmul(sp[:, 512:1024], lhsT=lhsT,
                                     rhs=emba[:, 512:1024], start=True, stop=True)
                    m8 = tops[:, 8 * i:8 * i + 8]
                    nc.vector.max(out=m8, in_=sp[:])
                    idx8 = idxp.tile([128, 8], mybir.dt.uint32)
                    nc.vector.max_index(out=idx8[:], in_max=m8, in_values=sp[:])
                    g = gp.tile([128, D], f32)
                    nc.gpsimd.indirect_dma_start(
                        out=g[:], out_offset=None, in_=embg_d[:],
                        in_offset=bass.IndirectOffsetOnAxis(ap=idx8[:, 0:1], axis=0),
                    )
                    gt = pstp.tile([D, 128], f32)
                    nc.tensor.transpose(out=gt[:], in_=g[:], identity=ident[:])
                    nc.scalar.copy(out=stage[:, t * 128:(t + 1) * 128], in_=gt[:])
                nc.sync.dma_start(quant_d[b], stage[:])


def get_program(repeat=1):
    key = ("nc", repeat)
    if key not in _CACHE:
        _CACHE[key] = _build_program(repeat)
    return _CACHE[key]


def make_in_maps(x, emb):
    """Host-side shard/prep. x [64,64,32,32] f32, emb [1024,64] f32."""
    x = np.ascontiguousarray(np.asarray(x, dtype=np.float32))
    emb = np.ascontiguousarray(np.asarray(emb, dtype=np.float32))
    e_sq = (emb * emb).sum(axis=1)                     # fp32, like reference
    emba = np.concatenate([emb.T, (-0.5 * e_sq)[None, :]], axis=0)
    emba = np.ascontiguousarray(emba.astype(np.float32))
    xr = x.reshape(B, D, HW)
    return [
        {"x": np.ascontiguousarray(xr[c * BPC:(c + 1) * BPC]),
         "emba": emba, "embg": emb}
        for c in range(N_CORES)
    ]


def postprocess(results, x):
    quant = np.concatenate([r["quant"] for r in results], axis=0)
    quant = quant.reshape(B, D, H, W)
    msum = sum(np.sum(r["msum"], dtype=np.float64) for r in results)
    x_sq = np.sum(np.asarray(x, dtype=np.float64) ** 2)
    n_el = B * D * HW
    loss = np.float32((1.0 + COMMITMENT_COST) * (x_sq - 2.0 * msum) / n_el)
    return quant, loss


def kernel(x, emb):
    from concourse.bass_utils import run_bass_kernel_spmd
    nc = get_program()
    in_maps = make_in_maps(x, emb)
    res = run_bass_kernel_spmd(nc, in_maps, core_ids=list(range(N_CORES)))
    return postprocess(res.results, x)
